# revision 34
# baseline (speedup 1.0000x reference)
"""Trainium2 Bass kernel for Swin-style multi-head attention.

Problem: x[128,197,768] -> qkv -> 12-head attention with relative-position
bias -> proj. Data-parallel over batch across 8 NeuronCores (16 batches/core).

All matmuls run in bf16 (fp32 PSUM accumulation), which keeps the PE at the
full 1-column/cycle rate without float32r's >=256 moving-dim requirement, so
the attention matmuls shrink from 256 to 198 free columns. Measured output
error vs the fp32 reference is ~5e-3 (gate is 2e-2).

Per-core layout (16 batches, processed as 8 pairs):
  - xT [16, 768, 197] bf16 fed pre-transposed (and pre-converted) from host.
  - q,k computed feature-major [f, t] into one qk tile [128, 12, 2, 198]
    (ft 0-5 = q, 6-11 = k), written straight from PSUM by ACT (bf16 cast),
    query dim zero-padded 197->198 so DVE ops hit the 2x 16-bit mode.
  - v computed token-major in an augmented [t, 12, 65] layout whose 65th
    column is ones, so the AV matmul also emits softmax row-sums for free;
    PSUM->vt is a single strided ACT copy per half (no staging bounce).
  - scoresT[m, n] = k^T q for both heads of a pair into one PSUM tile
    [msz, 2, 198]; one ACT exp (scale=1/8) writes bf16; one DVE multiply
    applies the host-pre-exponentiated bias factor (2x mode, all bf16).
  - AV + rowsum from one matmul pair; normalization via DVE reciprocal,
    GPSIMD partition_broadcast, DVE multiply writing bf16 o_all.
  - proj consumes o_all [128, hp, b, n] with full 128-deep contraction;
    bias added by ACT (Identity + per-partition bias AP); DMA out per e-tile.

Schedule: each head-pair step emits its two qkv feature-tile matmul groups
(k then q) immediately followed by that head pair's score units, with
v-chunks, the previous pair's tail AVs, and the previous pair's projection
(split into 3-matmul chunks) woven between them as always-ready PE filler.
Spreading the score units across the whole pair keeps the PSUM-reuse WAR
(released by the ACT exp) off the PE critical path; the last two AV units
of each pair are carried into the next pair's qkv phase where the PE has a
deep independent backlog. Steady-state PE occupancy in the cost model is
~93% of the pure-matmul floor.
"""

import sys

import numpy as np

for _p in ('/opt/trn_rl_repo', '/root/.axon_site/_ro/trn_rl_repo'):
    if _p not in sys.path:
        sys.path.insert(0, _p)

B = 128
N = 197
NP = 198  # query free dim padded to even for DVE 2x 16-bit mode
C = 768
H = 12
DH = 64
SCALE = DH ** -0.5
NCORES = 8
BLOC = B // NCORES  # 16
M0, M1 = 128, N - 128  # key-dim tiles: 128 + 69


def build_nc(b_loc=BLOC, lin_r=True, attn_r=True, pdepth=6, reps=1):
    """Build the per-core Bass program (lin_r/attn_r kept for signature
    compatibility; the kernel is all-bf16)."""
    import concourse.bacc as bacc
    import concourse.tile as tile
    from concourse import library_config, mybir

    f32 = mybir.dt.float32
    bf16 = mybir.dt.bfloat16

    nc = bacc.Bacc("TRN2", target_bir_lowering=False, debug=False)
    xT = nc.dram_tensor("xT", [b_loc, C, N], bf16, kind="ExternalInput").ap()
    qkv_wT = nc.dram_tensor("qkv_wT", [C, 3 * C], bf16, kind="ExternalInput").ap()
    proj_wT = nc.dram_tensor("proj_wT", [C, C], bf16, kind="ExternalInput").ap()
    proj_bt = nc.dram_tensor("proj_bt", [128, 6], f32, kind="ExternalInput").ap()
    biasT = nc.dram_tensor("biasT", [H, N, NP], bf16, kind="ExternalInput").ap()
    outT = nc.dram_tensor("outT", [b_loc, C, N], f32, kind="ExternalOutput").ap()

    n_pairs = b_loc // 2

    with tile.TileContext(nc) as tc:
        with (
            tc.tile_pool(name="consts", bufs=1) as consts,
            tc.tile_pool(name="xtp", bufs=2) as xtp,
            tc.tile_pool(name="qkp", bufs=2) as qkp,
            tc.tile_pool(name="vtp", bufs=2) as vtp,
            tc.tile_pool(name="sap", bufs=2) as sap,
            tc.tile_pool(name="pup", bufs=pdepth + 3) as pup,
            tc.tile_pool(name="recp", bufs=2) as recp,
            tc.tile_pool(name="oallp", bufs=2) as oallp,
            tc.tile_pool(name="obp", bufs=2) as obp,
            tc.tile_pool(name="psbig", bufs=2, space="PSUM") as psbig,
            tc.tile_pool(name="pss", bufs=2, space="PSUM") as pss,
            tc.tile_pool(name="pso", bufs=2, space="PSUM") as pso,
        ):
            nc.gpsimd.load_library(library_config.attnmlp)

            qkvw_sb = consts.tile([128, 6, 3 * C], bf16)
            projw_sb = consts.tile([128, 6, C], bf16)
            projb_sb = consts.tile([128, 6], f32)
            bias0_sb = consts.tile([128, H, NP], bf16)
            bias1_sb = consts.tile([128, H, NP], bf16)
            bias_sb = (bias0_sb, bias1_sb)
            ones_sb = consts.tile([128, H], bf16)

            nc.sync.dma_start(qkvw_sb, qkv_wT.rearrange("(ct p) f -> p ct f", p=128))
            nc.sync.dma_start(projw_sb, proj_wT.rearrange("(hp p) e -> p hp e", p=128))
            nc.sync.dma_start(projb_sb, proj_bt)
            nc.sync.dma_start(bias0_sb, biasT[:, 0:M0, :].rearrange("h p n -> p h n"))
            nc.sync.dma_start(bias1_sb[:M1], biasT[:, M0:N, :].rearrange("h p n -> p h n"))
            nc.vector.memset(ones_sb, 1.0)

            prev_proj_chunks = []
            carry_avs = []
            for pp in range(reps * n_pairs):
                b0 = 2 * (pp % n_pairs)

                # ---- load x pair, feature-major, bf16 straight from HBM ----
                xt = xtp.tile([128, 6, 2, N], bf16)
                for b in (0, 1):
                    nc.sync.dma_start(
                        xt[:, :, b, :],
                        xT[b0 + b].rearrange("(ct p) n -> p ct n", p=128),
                    )

                # ---- q/k feature-major [f-tile, b, n], n zero-padded ----
                qk = qkp.tile([128, H, 2, NP], bf16)
                nc.vector.memset(qk[:, :, :, N:], 0.0)

                def ft_group(ft, qk=qk, xt=xt):
                    ps = psbig.tile([128, 2, N], f32, tag="mmbig")
                    for ct in range(6):
                        nc.tensor.matmul(
                            ps,
                            qkvw_sb[:, ct, ft * 128:(ft + 1) * 128],
                            xt[:, ct],
                            start=(ct == 0),
                            stop=(ct == 5),
                        )
                    nc.scalar.copy(out=qk[:, ft, :, :N], in_=ps)

                # ---- v token-major, augmented layout [t, 12, 65] ----
                vts = [
                    [vtp.tile([128, H, 65], bf16, tag=f"vt{b}{tci}",
                              name=f"vt{b}{tci}")
                     for tci in range(2)]
                    for b in (0, 1)
                ]

                emitted_halves = set()

                def v_half(b, tci, half, vts_=vts, xt=xt, eh=emitted_halves):
                    eh.add((b, tci, half))
                    t0, tsz = ((0, M0), (M0, M1))[tci]
                    vt = vts_[b][tci]
                    vt_r = vt.rearrange("p (g two) c -> p two g c", two=2)
                    psv = psbig.tile([128, 384], f32, tag="mmbig")
                    for ct in range(6):
                        nc.tensor.matmul(
                            psv[:tsz],
                            xt[:, ct, b, t0:t0 + tsz],
                            qkvw_sb[:, ct, 2 * C + half * 384:2 * C + (half + 1) * 384],
                            start=(ct == 0),
                            stop=(ct == 5),
                        )
                    psv_v = psv.rearrange("p (g two d) -> p two g d", two=2, d=64)
                    nc.scalar.copy(
                        out=vt_r[:tsz, :, half * 3:(half + 1) * 3, 0:64],
                        in_=psv_v[:tsz],
                    )
                    if half == 1:
                        nc.gpsimd.tensor_copy(out=vt[:tsz, :, 64], in_=ones_sb[:tsz])

                # ---- attention, software-pipelined over (batch, head-pair) ----
                o_all = oallp.tile([128, 6, 2, N], bf16)

                def emit_scores(b, hp):
                    """scores + exp + bias for both heads of pair hp.

                    The two heads' score matmuls run at different PE tile
                    positions ((0,0) and (64,0)); landing both in one PSUM
                    bank crashes real HW, so the pair tile spans TWO banks
                    ([128, 2, 512] f32) and the exp reads across them with
                    a strided AP.
                    """
                    h0 = 2 * hp
                    pus = []
                    for mt, (m0, msz) in enumerate(((0, M0), (M0, M1))):
                        ps = pss.tile([128, 2, 512], f32, tag="s")
                        for par in (0, 1):
                            p0 = 64 * par
                            nc.tensor.matmul(
                                ps[:msz, par, 0:NP],
                                qk[p0:p0 + 64, 6 + hp, b, m0:m0 + msz],
                                qk[p0:p0 + 64, hp, b, :],
                                start=True,
                                stop=True,
                            )
                        sa = sap.tile([128, 2, NP], bf16, tag=f"sa{mt}")
                        nc.scalar.activation(
                            out=sa[:msz], in_=ps[:msz, :, 0:NP],
                            func=mybir.ActivationFunctionType.Exp, scale=SCALE,
                        )
                        pu = pup.tile([128, 2, NP], bf16, tag=f"pu{mt}")
                        nc.vector.tensor_mul(
                            out=pu[:msz], in0=sa[:msz],
                            in1=bias_sb[mt][:msz, h0:h0 + 2, :],
                        )
                        pus.append(pu)
                    return pus

                def emit_av(b, hp, pus, vts_=vts, o_all=o_all, eh=emitted_halves):
                    assert len([1 for (bb, _, _) in eh if bb == b]) == 4, \
                        f"AV(b={b}) emitted before its v tiles"
                    h0 = 2 * hp
                    vt0, vt1 = vts_[b]
                    ps_o = pso.tile([128, 2, NP], f32, tag="opair")
                    for par, h in ((0, h0), (1, h0 + 1)):
                        nc.tensor.matmul(
                            ps_o[0:65, par, :], vt0[:, h, :],
                            pus[0][:, par, :], start=True, stop=False,
                        )
                        nc.tensor.matmul(
                            ps_o[0:65, par, :], vt1[:M1, h, :],
                            pus[1][:M1, par, :], start=False, stop=True,
                        )
                    rec = recp.tile([1, 2, NP], f32, tag="rec")
                    nc.vector.reciprocal(out=rec, in_=ps_o[64:65, :, :])
                    recb = recp.tile([64, 2, N], f32, tag="recb")
                    for par in (0, 1):
                        nc.gpsimd.partition_broadcast(
                            recb[:, par, :], rec[:, par, :N]
                        )
                        nc.vector.tensor_mul(
                            out=o_all[par * 64:par * 64 + 64, hp, b, :],
                            in0=ps_o[0:64, par, :N],
                            in1=recb[:, par, :],
                        )

                def make_proj_chunks(et, o_all_=o_all, b0_=b0):
                    """Split one proj e-tile into two 3-matmul chunks so a
                    chunk of ready PE work can be interleaved after EVERY
                    attention unit (hides the exp->mul latency of the
                    scores pipeline)."""
                    state = {}

                    def chunk_a():
                        psp = psbig.tile([128, 2, N], f32, tag="mmbig", name=f"psp{et}")
                        state['psp'] = psp
                        for hp in range(3):
                            nc.tensor.matmul(
                                psp,
                                projw_sb[:, hp, et * 128:(et + 1) * 128],
                                o_all_[:, hp],
                                start=(hp == 0),
                                stop=False,
                            )

                    def chunk_b():
                        psp = state['psp']
                        for hp in range(3, 6):
                            nc.tensor.matmul(
                                psp,
                                projw_sb[:, hp, et * 128:(et + 1) * 128],
                                o_all_[:, hp],
                                start=False,
                                stop=(hp == 5),
                            )
                        ob = obp.tile([128, 2, N], f32, tag="ob", name=f"ob{et}")
                        nc.scalar.activation(
                            out=ob, in_=psp,
                            func=mybir.ActivationFunctionType.Identity,
                            bias=projb_sb[:, et:et + 1],
                        )
                        nc.sync.dma_start(
                            outT[b0_:b0_ + 2, et * 128:(et + 1) * 128, :].rearrange(
                                "b p n -> p b n"
                            ),
                            ob,
                        )
                    return [chunk_a, chunk_b]

                # Interleaved schedule: each hp step emits its two ft groups
                # (k then q) followed by the two (b, hp) score units, with
                # v-halves / carried AVs / prev-pair proj chunks as PE filler
                # between them. Spreading the score units across the whole
                # pair keeps consecutive units' PSUM-reuse WAR (released by
                # the ACT exp) off the PE critical path.
                fillers = list(carry_avs)
                fillers += [
                    (lambda b=b, tci=tci, half=half: v_half(b, tci, half))
                    for b in (0, 1) for tci in range(2) for half in range(2)
                ]
                fillers += prev_proj_chunks

                def pop_filler(k):
                    for _ in range(k):
                        if fillers:
                            fillers.pop(0)()

                pending = []

                def maybe_av(limit):
                    if len(pending) > limit:
                        pb, php, ppus = pending.pop(0)
                        emit_av(pb, php, ppus)

                for hp in range(6):
                    ft_group(6 + hp)  # k features for this head pair
                    ft_group(hp)      # q features
                    pop_filler(1)
                    pending.append((0, hp, emit_scores(0, hp)))
                    pop_filler(1)
                    maybe_av(pdepth)
                    pending.append((1, hp, emit_scores(1, hp)))
                    pop_filler(1)
                    maybe_av(pdepth)
                while len(pending) > 2:
                    pop_filler(1)
                    maybe_av(2)
                pop_filler(len(fillers))
                carry_avs = [
                    (lambda pb=pb, php=php, ppus=ppus, f=emit_av: f(pb, php, ppus))
                    for pb, php, ppus in pending
                ]
                pending = []
                prev_proj_chunks = [
                    c for et in range(6) for c in make_proj_chunks(et)
                ]

            # ---- tail: remaining AVs and final pair's proj ----
            for u in carry_avs:
                u()
            for u in prev_proj_chunks:
                u()
    nc.compile()
    return nc


def prep_inputs(x, qkv_w, proj_w, proj_b, bias_table, rel_idx):
    """Host-side data prep shared by kernel() and test harness."""
    import ml_dtypes

    bf = ml_dtypes.bfloat16
    x = np.asarray(x, np.float32)
    qkv_w = np.asarray(qkv_w, np.float32)
    proj_w = np.asarray(proj_w, np.float32)
    proj_b = np.asarray(proj_b, np.float32)
    bias_table = np.asarray(bias_table, np.float32)
    rel_idx = np.asarray(rel_idx)

    xT = np.ascontiguousarray(
        x.reshape(NCORES, BLOC, N, C).transpose(0, 1, 3, 2)
    ).astype(bf)
    qkv_wT = np.ascontiguousarray(qkv_w.T).astype(bf)
    proj_wT = np.ascontiguousarray(proj_w.T).astype(bf)
    proj_bt = np.ascontiguousarray(proj_b.reshape(6, 128).T)
    bias_full = bias_table[rel_idx]  # [n, m, h]
    biasT = np.ones((H, N, NP), np.float32)
    biasT[:, :, :N] = np.exp(bias_full.transpose(2, 1, 0))
    biasT = biasT.astype(bf)
    return xT, qkv_wT, proj_wT, proj_bt, biasT


_NC_CACHE = {}


def _get_nc(**kw):
    key = tuple(sorted(kw.items()))
    if key not in _NC_CACHE:
        _NC_CACHE[key] = build_nc(**kw)
    return _NC_CACHE[key]


def kernel(x, qkv_w, proj_w, proj_b, bias_table, rel_idx,
           _lin_r=True, _attn_r=True, _trace=False):
    from concourse.bass_utils import run_bass_kernel_spmd

    xT, qkv_wT, proj_wT, proj_bt, biasT = prep_inputs(
        x, qkv_w, proj_w, proj_b, bias_table, rel_idx
    )
    nc = _get_nc(lin_r=_lin_r, attn_r=_attn_r)
    in_maps = [
        {
            "xT": np.ascontiguousarray(xT[c]),
            "qkv_wT": qkv_wT,
            "proj_wT": proj_wT,
            "proj_bt": proj_bt,
            "biasT": biasT,
        }
        for c in range(NCORES)
    ]
    res = run_bass_kernel_spmd(nc, in_maps, list(range(NCORES)), trace=_trace)
    outs = np.stack([res.results[c]["outT"] for c in range(NCORES)])  # [8,16,768,197]
    out = outs.reshape(B, C, N).transpose(0, 2, 1)
    out = np.ascontiguousarray(out, np.float32)
    if _trace:
        return out, res
    return out
